# revision 12
# baseline (speedup 1.0000x reference)
"""Trainium2 Bass kernel for nn_Attention_39676907884025.

Reference semantics: q_param (a scalar) is broadcast over both query and key,
so the score matrix qk[b,q,k] = sum_d p*p is CONSTANT along the softmax axis.
Softmax of a constant row is exactly uniform (x - max(x) == 0 bit-exactly,
exp(0) == 1, sum == SK exactly, 1/SK is a power of two), so

    out[b, q, :] = (1/SK) * sum_k value[b, k, :]     for every q.

query / key / q_param never need to touch the device.

Distribution: data-parallel over batch B=16 across 8 NeuronCores (2 batches
per core). Per core the kernel is pure HBM streaming: read value (2MB),
write out (2MB); HBM-per-NC is ~358 GB/s, so the data floor is ~11.3us.

v3 (raw bacc, minimal program): three DMA queues (sync+scalar HWDGE,
gpsimd SWDGE), each warmed with a tiny first DMA (SWDGE otherwise has a
~3us cold start). Loads use the xt[p, t*128+d] = V[p*16+t, d] layout split
5/6/5 row-tiles per queue per batch. DVE tensor_reduces the sync/scalar
chunks as they land; gpsimd tensor_reduces its own chunk; two DVE adds
fold the three partials, the last one casting to bf16. One single-pass
bf16 matmul per batch against a constant 1/2048 stationary tile reduces
across partitions and broadcasts to all 128 rows; ACT copies PSUM out and
replicates to a 512-col wide tile; stores (2KB elems) go out 6/4/6 tiles
per queue, b0's stores overlapping b1's loads.

The profiler executes the NEFF twice, so every semaphore is cleared at a
provably-quiescent point at the end of each execution (each DMA queue
clears its own sems after its store-drain wait; vector clears the compute
sems after a 3-way s_done rendezvous). Without this, execution 2 starts
with stale semaphore values and races.
"""

import os
import sys

import numpy as np

if "/opt/trn_rl_repo" not in sys.path:
    sys.path.insert(0, "/opt/trn_rl_repo")

B, SQ, SK, D, DV = 16, 2048, 2048, 128, 128
N_CORES = 8
BPC = B // N_CORES  # batches per core
P = 128

LAST_RESULT = None  # BassKernelResults of the most recent run (for profiling)


def _build_nc_v3():
    import concourse.bacc as bacc
    import concourse.mybir as mybir

    WARM = os.environ.get("V3_WARM", "1") == "1"
    REDUCE = os.environ.get("V3_REDUCE", "1") == "1"
    BF16 = os.environ.get("V3_BF16", "1") == "1"
    ACT = os.environ.get("V3_ACT", "1") == "1"
    CLEAR = os.environ.get("V3_CLEAR", "1") == "1"

    f32 = mybir.dt.float32
    bf16 = mybir.dt.bfloat16 if BF16 else mybir.dt.float32
    nc = bacc.Bacc("TRN2", target_bir_lowering=False)

    val = nc.dram_tensor("value", [BPC, SK, DV], f32, kind="ExternalInput")
    out = nc.dram_tensor("out", [BPC, SQ, DV], f32, kind="ExternalOutput")

    w = nc.alloc_sbuf_tensor("w_const", [P, P], bf16)
    warm = nc.alloc_sbuf_tensor("warm", [4, DV], f32)
    xts = [nc.alloc_sbuf_tensor(f"xt{b}", [P, SK], f32) for b in range(BPC)]
    rSs = [nc.alloc_sbuf_tensor(f"rS{b}", [P, P], f32) for b in range(BPC)]
    rAs = [nc.alloc_sbuf_tensor(f"rA{b}", [P, P], f32) for b in range(BPC)]
    rGs = [nc.alloc_sbuf_tensor(f"rG{b}", [P, P], f32) for b in range(BPC)]
    sc1s = [nc.alloc_sbuf_tensor(f"sc1{b}", [P, 384], f32) for b in range(BPC)]
    cs = [nc.alloc_sbuf_tensor(f"c{b}", [P, P], f32) for b in range(BPC)]
    t4s = [nc.alloc_sbuf_tensor(f"t4_{b}", [P, P], bf16) for b in range(BPC)]
    wides = [nc.alloc_sbuf_tensor(f"wide{b}", [P, 512], f32) for b in range(BPC)]
    pss = [nc.alloc_psum_tensor(f"ps{b}", [P, P], f32) for b in range(BPC)]

    s_ls = nc.alloc_semaphore("s_ls")
    s_la = nc.alloc_semaphore("s_la")
    s_lg = nc.alloc_semaphore("s_lg")
    s_dve = nc.alloc_semaphore("s_dve")
    s_mm = nc.alloc_semaphore("s_mm")
    s_wide = nc.alloc_semaphore("s_wide")
    s_ss = nc.alloc_semaphore("s_ss")
    s_sa = nc.alloc_semaphore("s_sa")
    s_sg = nc.alloc_semaphore("s_sg")
    # cleared (gpsimd, start of every execution) in upstream-first order:
    # compute sems first so no in-flight increment can land after its clear
    _ALL_SEMS = [s_mm, s_dve, s_wide, s_ls, s_la, s_lg, s_ss, s_sa, s_sg]

    WOFF = 16 if WARM else 0  # warm DMA shifts load-sem thresholds

    # load tile split per batch: sync t[0:5), scalar t[5:11), gp t[11:16)
    # store tile split per batch: sync t[0:6), scalar t[6:10), gp t[10:16)

    def xsb(b):
        return xts[b][:].rearrange("p (t d) -> p t d", d=DV)

    def xdr(b):
        return val[b].rearrange("(p t) d -> p t d", p=P)

    def odr(b):
        return out[b].rearrange("(p t) d -> p t d", p=P)

    def wq(b):
        return wides[b][:].rearrange("p (q d) -> p q d", d=DV)

    def load(eng, b, t0, t1, sem):
        return eng.dma_start(
            xsb(b)[:, t0:t1, :], xdr(b)[:, t0:t1, :]
        ).then_inc(sem, 16)

    def chunk_view(b, t0, t1):
        return (
            xts[b][:, t0 * DV : t1 * DV]
            .rearrange("p (t d) -> p d t", d=DV)
        )

    with nc.Block() as block:

        @block.sync
        def _(sync):
            if WARM:
                sync.dma_start(warm[0:1, :], val[0, 0:1, :]).then_inc(s_ls, 16)
            load(sync, 0, 0, 5, s_ls)
            load(sync, 1, 0, 5, s_ls)
            sync.wait_ge(s_wide, 1)
            sync.dma_start(odr(0)[:, 0:4, :], wq(0)).then_inc(s_ss, 16)
            sync.dma_start(odr(0)[:, 4:6, :], wq(0)[:, 0:2, :]).then_inc(s_ss, 16)
            sync.wait_ge(s_wide, 2)
            sync.dma_start(odr(1)[:, 0:4, :], wq(1)).then_inc(s_ss, 16)
            sync.dma_start(odr(1)[:, 4:6, :], wq(1)[:, 0:2, :]).then_inc(s_ss, 16)
            sync.wait_ge(s_ss, 64)

        @block.scalar
        def _(scalar):
            if WARM:
                scalar.dma_start(warm[1:2, :], val[0, 1:2, :]).then_inc(s_la, 16)
            load(scalar, 0, 5, 11, s_la)
            load(scalar, 1, 5, 11, s_la)
            for b in range(BPC):
                if ACT:
                    scalar.wait_ge(s_mm, b + 1)
                    scalar.copy(wides[b][:, 0:P], pss[b][:])
                    scalar.copy(wides[b][:, P : 2 * P], wides[b][:, 0:P])
                    scalar.copy(
                        wides[b][:, 2 * P : 4 * P], wides[b][:, 0 : 2 * P]
                    ).then_inc(s_wide, 1)
                else:
                    scalar.wait_ge(s_wide, b + 1)
                scalar.dma_start(odr(b)[:, 6:10, :], wq(b)).then_inc(s_sa, 16)
            scalar.wait_ge(s_sa, 32)

        @block.gpsimd
        def _(gpsimd):
            if WARM:
                gpsimd.dma_start(warm[2:3, :], val[0, 2:3, :]).then_inc(s_lg, 16)
            if CLEAR:
                for s in _ALL_SEMS:
                    gpsimd.sem_clear(s)
            load(gpsimd, 0, 11, 16, s_lg)
            load(gpsimd, 1, 11, 16, s_lg)
            for b in range(BPC):
                gpsimd.wait_ge(s_wide, b + 1)
                gpsimd.dma_start(
                    odr(b)[:, 10:14, :], wq(b)
                ).then_inc(s_sg, 16)
                gpsimd.dma_start(
                    odr(b)[:, 14:16, :], wq(b)[:, 0:2, :]
                ).then_inc(s_sg, 16)
            gpsimd.wait_ge(s_sg, 64)

        @block.vector
        def _(vector):
            vector.memset(w[:], 1.0 / SK)

            def red(dst, b, t0, t1):
                if REDUCE:
                    vector.tensor_reduce(
                        dst[:],
                        chunk_view(b, t0, t1),
                        axis=mybir.AxisListType.X,
                        op=mybir.AluOpType.add,
                    )
                else:
                    x = xts[b]
                    lo, n = t0 * DV, t1 - t0
                    s1 = sc1s[b]
                    if n == 5:
                        vector.tensor_add(
                            s1[:, 0:256], x[:, lo : lo + 256],
                            x[:, lo + 256 : lo + 512],
                        )
                        vector.tensor_add(
                            dst[:], s1[:, 0:128], s1[:, 128:256]
                        )
                        vector.tensor_add(
                            dst[:], dst[:], x[:, lo + 512 : lo + 640]
                        )
                    else:  # n == 6
                        vector.tensor_add(
                            s1[:, 0:384], x[:, lo : lo + 384],
                            x[:, lo + 384 : lo + 768],
                        )
                        vector.tensor_add(
                            dst[:], s1[:, 0:128], s1[:, 128:256]
                        )
                        vector.tensor_add(
                            dst[:], dst[:], s1[:, 256:384]
                        )

            for b in range(BPC):
                th = WOFF + 16 * (b + 1)
                vector.wait_ge(s_ls, th)
                red(rSs[b], b, 0, 5)
                vector.wait_ge(s_la, th)
                red(rAs[b], b, 5, 11)
                vector.tensor_add(cs[b][:], rSs[b][:], rAs[b][:])
                vector.wait_ge(s_lg, th)
                red(rGs[b], b, 11, 16)
                vector.tensor_add(t4s[b][:], cs[b][:], rGs[b][:]).then_inc(
                    s_dve, 1
                )
                if not ACT:
                    vector.wait_ge(s_mm, b + 1)
                    vector.tensor_copy(wides[b][:, 0:P], pss[b][:])
                    vector.tensor_copy(wides[b][:, P : 2 * P], wides[b][:, 0:P])
                    vector.tensor_copy(
                        wides[b][:, 2 * P : 4 * P], wides[b][:, 0 : 2 * P]
                    ).then_inc(s_wide, 1)

        @block.tensor
        def _(tensor):
            # w readiness rides on s_dve: vector memsets w before its reduces
            for b in range(BPC):
                tensor.wait_ge(s_dve, b + 1)
                nc.tensor.matmul(
                    pss[b][:], w[:], t4s[b][:], start=True, stop=True
                ).then_inc(s_mm, 1)

    nc.compile()
    return nc


def _build_nc_tile():
    """Tile-scheduled fallback (the 28.3us baseline)."""
    import concourse.bacc as bacc
    import concourse.mybir as mybir
    from concourse.tile import TileContext

    f32 = mybir.dt.float32
    nc = bacc.Bacc("TRN2", target_bir_lowering=False)

    val = nc.dram_tensor("value", [BPC, SK, DV], f32, kind="ExternalInput")
    out = nc.dram_tensor("out", [BPC, SQ, DV], f32, kind="ExternalOutput")

    with TileContext(nc) as tc:
        with (
            tc.tile_pool(name="x", bufs=3) as xpool,
            tc.tile_pool(name="tree", bufs=3) as tpool,
            tc.tile_pool(name="const", bufs=1) as cpool,
            tc.tile_pool(name="psum", bufs=4, space="PSUM") as ppool,
        ):
            w = cpool.tile([P, P], f32)
            nc.vector.memset(w[:], 1.0 / SK)
            dma_eng = [nc.sync, nc.scalar]

            for b in range(BPC):
                xt = xpool.tile([P, SK], f32)
                xdst = xt[:].rearrange("p (t d) -> p t d", d=DV)
                xsrc = val[b].rearrange("(p t) d -> p t d", p=P)

                accs = []
                for qi in range(4):
                    t0, t1 = 4 * qi, 4 * (qi + 1)
                    dma_eng[qi % 2].dma_start(
                        xdst[:, t0:t1, :], xsrc[:, t0:t1, :]
                    )
                    lo, hi = 512 * qi, 512 * (qi + 1)
                    a = tpool.tile([P, 256], f32, tag=f"a{qi % 2}")
                    nc.vector.tensor_add(
                        a[:], xt[:, lo : lo + 256], xt[:, lo + 256 : hi]
                    )
                    acc = tpool.tile([P, P], f32, tag=f"acc{qi}")
                    nc.vector.tensor_add(acc[:], a[:, 0:128], a[:, 128:256])
                    accs.append(acc)

                s01 = tpool.tile([P, P], f32, tag="s01")
                nc.vector.tensor_add(s01[:], accs[0][:], accs[1][:])
                s23 = tpool.tile([P, P], f32, tag="s23")
                nc.vector.tensor_add(s23[:], accs[2][:], accs[3][:])
                t4 = tpool.tile([P, P], f32, tag="t4")
                nc.vector.tensor_add(t4[:], s01[:], s23[:])

                ps = ppool.tile([P, P], f32)
                nc.tensor.matmul(ps[:], w[:], t4[:], start=True, stop=True)

                wide = xpool.tile([P, 512], f32, tag="wide")
                nc.vector.tensor_copy(wide[:, 0:P], ps[:])
                nc.vector.tensor_copy(wide[:, P : 2 * P], wide[:, 0:P])
                nc.vector.tensor_copy(wide[:, 2 * P : 4 * P], wide[:, 0 : 2 * P])

                odst = out[b].rearrange("(p t) d -> p t d", p=P)
                wsrc = wide[:].rearrange("p (t d) -> p t d", d=DV)
                for qi in range(4):
                    t0, t1 = 4 * qi, 4 * (qi + 1)
                    dma_eng[qi % 2].dma_start(odst[:, t0:t1, :], wsrc)

    nc.compile()
    return nc


_BUILDERS = {"raw3": _build_nc_v3, "tile": _build_nc_tile}
KERNEL_VARIANT = os.environ.get("BASS_VARIANT", "raw3")


def kernel(query=None, key=None, value=None, q_param=None, _trace=False):
    from concourse.bass_utils import run_bass_kernel_spmd

    global LAST_RESULT

    value = np.ascontiguousarray(np.asarray(value, dtype=np.float32))
    assert value.shape == (B, SK, DV), value.shape

    nc = _BUILDERS[KERNEL_VARIANT]()
    shards = value.reshape(N_CORES, BPC, SK, DV)
    in_maps = [{"value": shards[i]} for i in range(N_CORES)]

    LAST_RESULT = run_bass_kernel_spmd(
        nc, in_maps, list(range(N_CORES)), trace=_trace
    )
    return np.concatenate(
        [LAST_RESULT.results[i]["out"] for i in range(N_CORES)], axis=0
    )


# revision 18
# speedup vs baseline: 1.0768x; 1.0768x over previous
"""Trainium2 Bass kernel for nn_Attention_39676907884025.

Reference semantics: q_param (a scalar) is broadcast over both query and key,
so the score matrix qk[b,q,k] = sum_d p*p is CONSTANT along the softmax axis.
Softmax of a constant row is exactly uniform (x - max(x) == 0 bit-exactly,
exp(0) == 1, sum == SK exactly, 1/SK is a power of two), so

    out[b, q, :] = (1/SK) * sum_k value[b, k, :]     for every q.

query / key / q_param never need to touch the device.

Distribution: data-parallel over batch B=16 across 8 NeuronCores (2 batches
per core). Per core the kernel is pure HBM streaming: read value (2MB),
write out (2MB); HBM-per-NC is ~358 GB/s, so the data floor is ~11.3us.

v3 (raw bacc, minimal program): three DMA queues (sync+scalar HWDGE,
gpsimd SWDGE), each warmed with a tiny first DMA (SWDGE otherwise has a
~3us cold start). Loads use the xt[p, t*128+d] = V[p*16+t, d] layout split
5/6/5 row-tiles per queue per batch. DVE tensor_reduces the sync/scalar
chunks as they land; gpsimd tensor_reduces its own chunk; two DVE adds
fold the three partials, the last one casting to bf16. One single-pass
bf16 matmul per batch against a constant 1/2048 stationary tile reduces
across partitions and broadcasts to all 128 rows; ACT copies PSUM out and
replicates to a 512-col wide tile; stores (2KB elems) go out 6/4/6 tiles
per queue, b0's stores overlapping b1's loads.

The profiler executes the NEFF twice, so every semaphore is cleared at a
provably-quiescent point at the end of each execution (each DMA queue
clears its own sems after its store-drain wait; vector clears the compute
sems after a 3-way s_done rendezvous). Without this, execution 2 starts
with stale semaphore values and races.
"""

import os
import sys

import numpy as np

if "/opt/trn_rl_repo" not in sys.path:
    sys.path.insert(0, "/opt/trn_rl_repo")

B, SQ, SK, D, DV = 16, 2048, 2048, 128, 128
N_CORES = 8
BPC = B // N_CORES  # batches per core
P = 128

LAST_RESULT = None  # BassKernelResults of the most recent run (for profiling)


def _build_nc_v3():
    import concourse.bacc as bacc
    import concourse.mybir as mybir

    WARM = os.environ.get("V3_WARM", "1") == "1"
    REDUCE = os.environ.get("V3_REDUCE", "1") == "1"
    BF16 = os.environ.get("V3_BF16", "1") == "1"
    ACT = os.environ.get("V3_ACT", "1") == "1"
    CLEAR = os.environ.get("V3_CLEAR", "1") == "1"

    f32 = mybir.dt.float32
    bf16 = mybir.dt.bfloat16 if BF16 else mybir.dt.float32
    nc = bacc.Bacc("TRN2", target_bir_lowering=False)

    val = nc.dram_tensor("value", [BPC, SK, DV], f32, kind="ExternalInput")
    out = nc.dram_tensor("out", [BPC, SQ, DV], f32, kind="ExternalOutput")

    w = nc.alloc_sbuf_tensor("w_const", [P, P], bf16)
    warm = nc.alloc_sbuf_tensor("warm", [4, DV], f32)
    xts = [nc.alloc_sbuf_tensor(f"xt{b}", [P, SK], f32) for b in range(BPC)]
    rSs = [nc.alloc_sbuf_tensor(f"rS{b}", [P, P], f32) for b in range(BPC)]
    rAs = [nc.alloc_sbuf_tensor(f"rA{b}", [P, P], f32) for b in range(BPC)]
    rGs = [nc.alloc_sbuf_tensor(f"rG{b}", [P, P], f32) for b in range(BPC)]
    sc1s = [nc.alloc_sbuf_tensor(f"sc1{b}", [P, 384], f32) for b in range(BPC)]
    cs = [nc.alloc_sbuf_tensor(f"c{b}", [P, P], f32) for b in range(BPC)]
    t4s = [nc.alloc_sbuf_tensor(f"t4_{b}", [P, P], bf16) for b in range(BPC)]
    wides = [nc.alloc_sbuf_tensor(f"wide{b}", [P, 512], f32) for b in range(BPC)]
    pss = [nc.alloc_psum_tensor(f"ps{b}", [P, P], f32) for b in range(BPC)]

    s_ls = nc.alloc_semaphore("s_ls")
    s_la = nc.alloc_semaphore("s_la")
    s_lg = nc.alloc_semaphore("s_lg")
    s_dve = nc.alloc_semaphore("s_dve")
    s_mm = nc.alloc_semaphore("s_mm")
    s_wide = nc.alloc_semaphore("s_wide")
    s_ss = nc.alloc_semaphore("s_ss")
    s_sa = nc.alloc_semaphore("s_sa")
    s_sg = nc.alloc_semaphore("s_sg")
    # cleared (gpsimd, start of every execution) in upstream-first order:
    # compute sems first so no in-flight increment can land after its clear
    _ALL_SEMS = [s_mm, s_dve, s_wide, s_ls, s_la, s_lg, s_ss, s_sa, s_sg]

    WOFF = 16 if WARM else 0  # warm DMA shifts load-sem thresholds

    # load tile split per batch: sync t[0:5), scalar t[5:11), gp t[11:16)
    # store tile split per batch: sync t[0:6), scalar t[6:10), gp t[10:16)

    def xsb(b):
        return xts[b][:].rearrange("p (t d) -> p t d", d=DV)

    def xdr(b):
        return val[b].rearrange("(p t) d -> p t d", p=P)

    def odr(b):
        return out[b].rearrange("(p t) d -> p t d", p=P)

    def wq(b):
        return wides[b][:].rearrange("p (q d) -> p q d", d=DV)

    def load(eng, b, t0, t1, sem):
        return eng.dma_start(
            xsb(b)[:, t0:t1, :], xdr(b)[:, t0:t1, :]
        ).then_inc(sem, 16)

    def chunk_view(b, t0, t1):
        return (
            xts[b][:, t0 * DV : t1 * DV]
            .rearrange("p (t d) -> p d t", d=DV)
        )

    with nc.Block() as block:

        @block.sync
        def _(sync):
            if WARM:
                sync.dma_start(warm[0:1, :], val[0, 0:1, :]).then_inc(s_ls, 16)
            load(sync, 0, 0, 5, s_ls)
            load(sync, 1, 0, 5, s_ls)
            sync.wait_ge(s_wide, 1)
            sync.dma_start(odr(0)[:, 0:4, :], wq(0)).then_inc(s_ss, 16)
            sync.dma_start(odr(0)[:, 4:6, :], wq(0)[:, 0:2, :]).then_inc(s_ss, 16)
            sync.wait_ge(s_wide, 2)
            sync.dma_start(odr(1)[:, 0:4, :], wq(1)).then_inc(s_ss, 16)
            sync.dma_start(odr(1)[:, 4:6, :], wq(1)[:, 0:2, :]).then_inc(s_ss, 16)
            sync.wait_ge(s_ss, 64)

        @block.scalar
        def _(scalar):
            if WARM:
                scalar.dma_start(warm[1:2, :], val[0, 1:2, :]).then_inc(s_la, 16)
            load(scalar, 0, 5, 11, s_la)
            load(scalar, 1, 5, 11, s_la)
            for b in range(BPC):
                if ACT:
                    scalar.wait_ge(s_mm, b + 1)
                    scalar.copy(wides[b][:, 0:P], pss[b][:])
                    scalar.copy(wides[b][:, P : 2 * P], wides[b][:, 0:P])
                    scalar.copy(
                        wides[b][:, 2 * P : 4 * P], wides[b][:, 0 : 2 * P]
                    ).then_inc(s_wide, 1)
                else:
                    scalar.wait_ge(s_wide, b + 1)
                scalar.dma_start(odr(b)[:, 6:10, :], wq(b)).then_inc(s_sa, 16)
            scalar.wait_ge(s_sa, 32)

        @block.gpsimd
        def _(gpsimd):
            if WARM:
                gpsimd.dma_start(warm[2:3, :], val[0, 2:3, :]).then_inc(s_lg, 16)
            if CLEAR:
                for s in _ALL_SEMS:
                    gpsimd.sem_clear(s)
            load(gpsimd, 0, 11, 16, s_lg)
            load(gpsimd, 1, 11, 16, s_lg)
            for b in range(BPC):
                gpsimd.wait_ge(s_wide, b + 1)
                gpsimd.dma_start(
                    odr(b)[:, 10:14, :], wq(b)
                ).then_inc(s_sg, 16)
                gpsimd.dma_start(
                    odr(b)[:, 14:16, :], wq(b)[:, 0:2, :]
                ).then_inc(s_sg, 16)
            gpsimd.wait_ge(s_sg, 64)

        @block.vector
        def _(vector):
            vector.memset(w[:], 1.0 / SK)

            def red(dst, b, t0, t1):
                if REDUCE:
                    vector.tensor_reduce(
                        dst[:],
                        chunk_view(b, t0, t1),
                        axis=mybir.AxisListType.X,
                        op=mybir.AluOpType.add,
                    )
                else:
                    x = xts[b]
                    lo, n = t0 * DV, t1 - t0
                    s1 = sc1s[b]
                    if n == 5:
                        vector.tensor_add(
                            s1[:, 0:256], x[:, lo : lo + 256],
                            x[:, lo + 256 : lo + 512],
                        )
                        vector.tensor_add(
                            dst[:], s1[:, 0:128], s1[:, 128:256]
                        )
                        vector.tensor_add(
                            dst[:], dst[:], x[:, lo + 512 : lo + 640]
                        )
                    else:  # n == 6
                        vector.tensor_add(
                            s1[:, 0:384], x[:, lo : lo + 384],
                            x[:, lo + 384 : lo + 768],
                        )
                        vector.tensor_add(
                            dst[:], s1[:, 0:128], s1[:, 128:256]
                        )
                        vector.tensor_add(
                            dst[:], dst[:], s1[:, 256:384]
                        )

            for b in range(BPC):
                th = WOFF + 16 * (b + 1)
                vector.wait_ge(s_ls, th)
                red(rSs[b], b, 0, 5)
                vector.wait_ge(s_la, th)
                red(rAs[b], b, 5, 11)
                vector.tensor_add(cs[b][:], rSs[b][:], rAs[b][:])
                vector.wait_ge(s_lg, th)
                red(rGs[b], b, 11, 16)
                vector.tensor_add(t4s[b][:], cs[b][:], rGs[b][:]).then_inc(
                    s_dve, 1
                )
                if not ACT:
                    vector.wait_ge(s_mm, b + 1)
                    vector.tensor_copy(wides[b][:, 0:P], pss[b][:])
                    vector.tensor_copy(wides[b][:, P : 2 * P], wides[b][:, 0:P])
                    vector.tensor_copy(
                        wides[b][:, 2 * P : 4 * P], wides[b][:, 0 : 2 * P]
                    ).then_inc(s_wide, 1)

        @block.tensor
        def _(tensor):
            # w readiness rides on s_dve: vector memsets w before its reduces
            for b in range(BPC):
                tensor.wait_ge(s_dve, b + 1)
                nc.tensor.matmul(
                    pss[b][:], w[:], t4s[b][:], start=True, stop=True
                ).then_inc(s_mm, 1)

    nc.compile()
    return nc


def _build_nc_v4():
    """2 HWDGE queues carry all data (SWDGE crawls at ~54-100 GB/s here);
    DVE folds all 16 row-tiles per batch (wide adds as chunks land, bf16
    cast on the last), one single-pass bf16 matmul per batch reduces
    across partitions and broadcasts; DVE replicates PSUM out to a
    512-col wide tile; each queue stores 2x4 tiles per batch.

    Semaphore discipline: a DMA's then_inc(sem, 16) arrives as +1 from each
    of the 16 SDMA engines, so with several DMAs on one semaphore a
    cumulative threshold can be met while an earlier DMA still has
    descriptors in flight on a lagging engine. Every waited-on load DMA
    therefore gets its OWN semaphore, waited at its full value (16); store
    semaphores are shared per queue because only the final total (64) is
    ever waited on. gpsimd zeroes all semaphores at the start of every
    execution (the profiler runs the NEFF twice; stale semaphores would
    race execution 2 -- and sem_inc/EventSemaphore updates from sync/scalar
    crash this runtime, so gpsimd sem_clear is the only proven mechanism).
    """
    import concourse.bacc as bacc
    import concourse.mybir as mybir

    f32 = mybir.dt.float32
    bf16 = mybir.dt.bfloat16
    nc = bacc.Bacc("TRN2", target_bir_lowering=False)

    val = nc.dram_tensor("value", [BPC, SK, DV], f32, kind="ExternalInput")
    out = nc.dram_tensor("out", [BPC, SQ, DV], f32, kind="ExternalOutput")

    w = nc.alloc_sbuf_tensor("w_const", [P, P], bf16)
    warm = nc.alloc_sbuf_tensor("warm", [4, DV], f32)
    xts = [nc.alloc_sbuf_tensor(f"xt{b}", [P, SK], f32) for b in range(BPC)]
    h1s = [nc.alloc_sbuf_tensor(f"h1_{b}", [P, 384], f32) for b in range(BPC)]
    h2s = [nc.alloc_sbuf_tensor(f"h2_{b}", [P, 384], f32) for b in range(BPC)]
    h3s = [nc.alloc_sbuf_tensor(f"h3_{b}", [P, 384], f32) for b in range(BPC)]
    h4s = [nc.alloc_sbuf_tensor(f"h4_{b}", [P, P], f32) for b in range(BPC)]
    h5s = [nc.alloc_sbuf_tensor(f"h5_{b}", [P, P], f32) for b in range(BPC)]
    pts = [nc.alloc_sbuf_tensor(f"pt{b}", [P, P], bf16) for b in range(BPC)]
    wides = [nc.alloc_sbuf_tensor(f"wide{b}", [P, 512], f32) for b in range(BPC)]
    pss = [nc.alloc_psum_tensor(f"ps{b}", [P, P], f32) for b in range(BPC)]

    s_wm = nc.alloc_semaphore("s_wm")  # warm DMAs; never waited on
    sA = [nc.alloc_semaphore(f"sA{b}") for b in range(BPC)]  # sync t[0:6)
    sB = [nc.alloc_semaphore(f"sB{b}") for b in range(BPC)]  # sync t[12:14)
    sC = [nc.alloc_semaphore(f"sC{b}") for b in range(BPC)]  # scalar t[6:12)
    sD = [nc.alloc_semaphore(f"sD{b}") for b in range(BPC)]  # scalar t[14:16)
    s_dve = nc.alloc_semaphore("s_dve")  # pt[b] ready
    s_mm = nc.alloc_semaphore("s_mm")    # ps[b] complete
    s_wide = nc.alloc_semaphore("s_wide")
    s_ss = nc.alloc_semaphore("s_ss")    # sync stores (final 64)
    s_sa = nc.alloc_semaphore("s_sa")    # scalar stores (final 64)
    _ALL_SEMS = (
        [s_mm, s_dve, s_wide] + sA + sB + sC + sD + [s_wm, s_ss, s_sa]
    )

    # stores: sync t[0:4) + t[4:8); scalar t[8:12) + t[12:16)

    def xsb(b):
        return xts[b][:].rearrange("p (t d) -> p t d", d=DV)

    def xdr(b):
        return val[b].rearrange("(p t) d -> p t d", p=P)

    def odr(b):
        return out[b].rearrange("(p t) d -> p t d", p=P)

    def wq(b):
        return wides[b][:].rearrange("p (q d) -> p q d", d=DV)

    def load(eng, b, t0, t1, sem):
        return eng.dma_start(
            xsb(b)[:, t0:t1, :], xdr(b)[:, t0:t1, :]
        ).then_inc(sem, 16)

    with nc.Block() as block:

        @block.sync
        def _(sync):
            sync.dma_start(warm[0:1, :], val[0, 0:1, :]).then_inc(s_wm, 16)
            load(sync, 0, 0, 6, sA[0])
            load(sync, 0, 12, 14, sB[0])
            load(sync, 1, 0, 6, sA[1])
            load(sync, 1, 12, 14, sB[1])
            for b in range(BPC):
                sync.wait_ge(s_wide, b + 1)
                sync.dma_start(odr(b)[:, 0:4, :], wq(b)).then_inc(s_ss, 16)
                sync.dma_start(odr(b)[:, 4:8, :], wq(b)).then_inc(s_ss, 16)
            sync.wait_ge(s_ss, 64)

        @block.scalar
        def _(scalar):
            scalar.dma_start(warm[1:2, :], val[0, 1:2, :]).then_inc(s_wm, 16)
            load(scalar, 0, 6, 12, sC[0])
            load(scalar, 0, 14, 16, sD[0])
            load(scalar, 1, 6, 12, sC[1])
            load(scalar, 1, 14, 16, sD[1])
            for b in range(BPC):
                scalar.wait_ge(s_wide, b + 1)
                scalar.dma_start(odr(b)[:, 8:12, :], wq(b)).then_inc(s_sa, 16)
                scalar.dma_start(odr(b)[:, 12:16, :], wq(b)).then_inc(s_sa, 16)
            scalar.wait_ge(s_sa, 64)

        @block.gpsimd
        def _(gpsimd):
            for s in _ALL_SEMS:
                gpsimd.sem_clear(s)

        @block.vector
        def _(vector):
            vector.memset(w[:], 1.0 / SK)
            for b in range(BPC):
                x = xts[b]
                vector.wait_ge(sA[b], 16)
                vector.tensor_add(h1s[b][:], x[:, 0:384], x[:, 384:768])
                vector.wait_ge(sC[b], 16)
                vector.tensor_add(h2s[b][:], x[:, 768:1152], x[:, 1152:1536])
                vector.tensor_add(h3s[b][:], h1s[b][:], h2s[b][:])
                vector.tensor_add(
                    h4s[b][:], h3s[b][:, 0:128], h3s[b][:, 128:256]
                )
                vector.tensor_add(h5s[b][:], h4s[b][:], h3s[b][:, 256:384])
                # tail tiles 12..15
                vector.wait_ge(sB[b], 16)
                vector.tensor_add(h4s[b][:], x[:, 1536:1664], x[:, 1664:1792])
                vector.tensor_add(h4s[b][:], h4s[b][:], h5s[b][:])
                vector.wait_ge(sD[b], 16)
                vector.tensor_add(h4s[b][:], h4s[b][:], x[:, 1792:1920])
                vector.tensor_add(
                    pts[b][:], h4s[b][:], x[:, 1920:2048]
                ).then_inc(s_dve, 1)
                vector.wait_ge(s_mm, b + 1)
                vector.tensor_copy(wides[b][:, 0:P], pss[b][:])
                vector.tensor_copy(wides[b][:, P : 2 * P], wides[b][:, 0:P])
                vector.tensor_copy(
                    wides[b][:, 2 * P : 4 * P], wides[b][:, 0 : 2 * P]
                ).then_inc(s_wide, 1)

        @block.tensor
        def _(tensor):
            # w readiness rides on s_dve: vector memsets w before its folds
            for b in range(BPC):
                tensor.wait_ge(s_dve, b + 1)
                nc.tensor.matmul(
                    pss[b][:], w[:], pts[b][:], start=True, stop=True
                ).then_inc(s_mm, 1)

    nc.compile()
    return nc


def _build_nc_v5():
    """Asymmetric 2-queue schedule around the ACT-queue ~3us cold start.

    Loads (per-DMA semaphores; see v4 docstring for why): sync(q1) carries
    b0 t[0:12) as 3x256KB plus b1 t[0:8) as 512KB; scalar(q10, cold) warms
    with a 512B dummy, then b0 t[12:16) and b1 t[8:16) as 2x256KB. DVE
    folds each batch with a 5-op wide tree (512/512/512/256/128-out, bf16
    cast on the last), one single-pass bf16 matmul reduces across
    partitions + broadcasts, ONE PSUM->SBUF copy, and each queue stores
    512KB per batch in a single DMA whose SBUF source is a stride-0
    broadcast AP over the 64KB mean tile (store descriptors pre-stage
    while loads drain, so the load->store transition is seamless).
    gpsimd zeroes all semaphores at execution start (profiler double-run).
    """
    import concourse.bacc as bacc
    import concourse.mybir as mybir

    BCAST = os.environ.get("V5_BCAST", "1") == "1"

    f32 = mybir.dt.float32
    bf16 = mybir.dt.bfloat16
    nc = bacc.Bacc("TRN2", target_bir_lowering=False)

    val = nc.dram_tensor("value", [BPC, SK, DV], f32, kind="ExternalInput")
    out = nc.dram_tensor("out", [BPC, SQ, DV], f32, kind="ExternalOutput")

    w = nc.alloc_sbuf_tensor("w_const", [P, P], bf16)
    warm = nc.alloc_sbuf_tensor("warm", [4, DV], f32)
    xts = [nc.alloc_sbuf_tensor(f"xt{b}", [P, SK], f32) for b in range(BPC)]
    h1s = [nc.alloc_sbuf_tensor(f"h1_{b}", [P, 512], f32) for b in range(BPC)]
    h2s = [nc.alloc_sbuf_tensor(f"h2_{b}", [P, 512], f32) for b in range(BPC)]
    h3s = [nc.alloc_sbuf_tensor(f"h3_{b}", [P, 512], f32) for b in range(BPC)]
    h4s = [nc.alloc_sbuf_tensor(f"h4_{b}", [P, 256], f32) for b in range(BPC)]
    pts = [nc.alloc_sbuf_tensor(f"pt{b}", [P, P], bf16) for b in range(BPC)]
    wides = [nc.alloc_sbuf_tensor(f"wide{b}", [P, 512], f32) for b in range(BPC)]
    pss = [nc.alloc_psum_tensor(f"ps{b}", [P, P], f32) for b in range(BPC)]

    s_wm = nc.alloc_semaphore("s_wm")  # warm DMA; never waited on
    sA = nc.alloc_semaphore("sA")  # q1 b0 t[0:4)
    sB = nc.alloc_semaphore("sB")  # q1 b0 t[4:8)
    sC = nc.alloc_semaphore("sC")  # q1 b0 t[8:12)
    sD = nc.alloc_semaphore("sD")  # q10 b0 t[12:16)
    sE = nc.alloc_semaphore("sE")  # q1 b1 t[0:8)
    sF = nc.alloc_semaphore("sF")  # q10 b1 t[8:12)
    sG = nc.alloc_semaphore("sG")  # q10 b1 t[12:16)
    s_dve = nc.alloc_semaphore("s_dve")
    s_mm = nc.alloc_semaphore("s_mm")
    s_wide = nc.alloc_semaphore("s_wide")
    s_ss = nc.alloc_semaphore("s_ss")  # sync stores (final 32)
    s_sa = nc.alloc_semaphore("s_sa")  # scalar stores (final 32)
    _ALL_SEMS = [
        s_mm, s_dve, s_wide, sA, sB, sC, sD, sE, sF, sG,
        s_wm, s_ss, s_sa,
    ]

    def xsb(b):
        return xts[b][:].rearrange("p (t d) -> p t d", d=DV)

    def xdr(b):
        return val[b].rearrange("(p t) d -> p t d", p=P)

    def odr(b):
        return out[b].rearrange("(p t) d -> p t d", p=P)

    def load(eng, b, t0, t1, sem):
        return eng.dma_start(
            xsb(b)[:, t0:t1, :], xdr(b)[:, t0:t1, :]
        ).then_inc(sem, 16)

    def store_src(b, nt):
        if BCAST:
            return (
                wides[b][:, 0:P]
                .rearrange("p (q d) -> p q d", d=DV)
                .broadcast_to([P, nt, DV])
            )
        return wides[b][:].rearrange("p (q d) -> p q d", d=DV)

    with nc.Block() as block:

        @block.sync
        def _(sync):
            load(sync, 0, 0, 4, sA)
            load(sync, 0, 4, 8, sB)
            load(sync, 0, 8, 12, sC)
            load(sync, 1, 0, 8, sE)
            for b in range(BPC):
                sync.wait_ge(s_wide, b + 1)
                sync.dma_start(
                    odr(b)[:, 0:8, :], store_src(b, 8)
                ).then_inc(s_ss, 16)
            sync.wait_ge(s_ss, 32)

        @block.scalar
        def _(scalar):
            scalar.dma_start(warm[1:2, :], val[0, 1:2, :]).then_inc(s_wm, 16)
            load(scalar, 0, 12, 16, sD)
            load(scalar, 1, 8, 12, sF)
            load(scalar, 1, 12, 16, sG)
            for b in range(BPC):
                scalar.wait_ge(s_wide, b + 1)
                scalar.dma_start(
                    odr(b)[:, 8:16, :], store_src(b, 8)
                ).then_inc(s_sa, 16)
            scalar.wait_ge(s_sa, 32)

        @block.gpsimd
        def _(gpsimd):
            for s in _ALL_SEMS:
                gpsimd.sem_clear(s)

        @block.vector
        def _(vector):
            vector.memset(w[:], 1.0 / SK)
            for b in range(BPC):
                x = xts[b]
                if b == 0:
                    vector.wait_ge(sA, 16)
                    vector.wait_ge(sB, 16)
                else:
                    vector.wait_ge(sE, 16)
                vector.tensor_add(h1s[b][:], x[:, 0:512], x[:, 512:1024])
                if b == 0:
                    vector.wait_ge(sC, 16)
                    vector.wait_ge(sD, 16)
                else:
                    vector.wait_ge(sF, 16)
                    vector.wait_ge(sG, 16)
                vector.tensor_add(h2s[b][:], x[:, 1024:1536], x[:, 1536:2048])
                vector.tensor_add(h3s[b][:], h1s[b][:], h2s[b][:])
                vector.tensor_add(
                    h4s[b][:], h3s[b][:, 0:256], h3s[b][:, 256:512]
                )
                vector.tensor_add(
                    pts[b][:], h4s[b][:, 0:128], h4s[b][:, 128:256]
                ).then_inc(s_dve, 1)
                vector.wait_ge(s_mm, b + 1)
                if BCAST:
                    vector.tensor_copy(
                        wides[b][:, 0:P], pss[b][:]
                    ).then_inc(s_wide, 1)
                else:
                    vector.tensor_copy(wides[b][:, 0:P], pss[b][:])
                    vector.tensor_copy(
                        wides[b][:, P : 2 * P], wides[b][:, 0:P]
                    )
                    vector.tensor_copy(
                        wides[b][:, 2 * P : 4 * P], wides[b][:, 0 : 2 * P]
                    ).then_inc(s_wide, 1)

        @block.tensor
        def _(tensor):
            # w readiness rides on s_dve: vector memsets w before its folds
            for b in range(BPC):
                tensor.wait_ge(s_dve, b + 1)
                nc.tensor.matmul(
                    pss[b][:], w[:], pts[b][:], start=True, stop=True
                ).then_inc(s_mm, 1)

    nc.compile()
    return nc


def _build_nc_tile():
    """Tile-scheduled fallback (the 28.3us baseline)."""
    import concourse.bacc as bacc
    import concourse.mybir as mybir
    from concourse.tile import TileContext

    f32 = mybir.dt.float32
    nc = bacc.Bacc("TRN2", target_bir_lowering=False)

    val = nc.dram_tensor("value", [BPC, SK, DV], f32, kind="ExternalInput")
    out = nc.dram_tensor("out", [BPC, SQ, DV], f32, kind="ExternalOutput")

    with TileContext(nc) as tc:
        with (
            tc.tile_pool(name="x", bufs=3) as xpool,
            tc.tile_pool(name="tree", bufs=3) as tpool,
            tc.tile_pool(name="const", bufs=1) as cpool,
            tc.tile_pool(name="psum", bufs=4, space="PSUM") as ppool,
        ):
            w = cpool.tile([P, P], f32)
            nc.vector.memset(w[:], 1.0 / SK)
            dma_eng = [nc.sync, nc.scalar]

            for b in range(BPC):
                xt = xpool.tile([P, SK], f32)
                xdst = xt[:].rearrange("p (t d) -> p t d", d=DV)
                xsrc = val[b].rearrange("(p t) d -> p t d", p=P)

                accs = []
                for qi in range(4):
                    t0, t1 = 4 * qi, 4 * (qi + 1)
                    dma_eng[qi % 2].dma_start(
                        xdst[:, t0:t1, :], xsrc[:, t0:t1, :]
                    )
                    lo, hi = 512 * qi, 512 * (qi + 1)
                    a = tpool.tile([P, 256], f32, tag=f"a{qi % 2}")
                    nc.vector.tensor_add(
                        a[:], xt[:, lo : lo + 256], xt[:, lo + 256 : hi]
                    )
                    acc = tpool.tile([P, P], f32, tag=f"acc{qi}")
                    nc.vector.tensor_add(acc[:], a[:, 0:128], a[:, 128:256])
                    accs.append(acc)

                s01 = tpool.tile([P, P], f32, tag="s01")
                nc.vector.tensor_add(s01[:], accs[0][:], accs[1][:])
                s23 = tpool.tile([P, P], f32, tag="s23")
                nc.vector.tensor_add(s23[:], accs[2][:], accs[3][:])
                t4 = tpool.tile([P, P], f32, tag="t4")
                nc.vector.tensor_add(t4[:], s01[:], s23[:])

                ps = ppool.tile([P, P], f32)
                nc.tensor.matmul(ps[:], w[:], t4[:], start=True, stop=True)

                wide = xpool.tile([P, 512], f32, tag="wide")
                nc.vector.tensor_copy(wide[:, 0:P], ps[:])
                nc.vector.tensor_copy(wide[:, P : 2 * P], wide[:, 0:P])
                nc.vector.tensor_copy(wide[:, 2 * P : 4 * P], wide[:, 0 : 2 * P])

                odst = out[b].rearrange("(p t) d -> p t d", p=P)
                wsrc = wide[:].rearrange("p (t d) -> p t d", d=DV)
                for qi in range(4):
                    t0, t1 = 4 * qi, 4 * (qi + 1)
                    dma_eng[qi % 2].dma_start(odst[:, t0:t1, :], wsrc)

    nc.compile()
    return nc


_BUILDERS = {"raw3": _build_nc_v3, "raw4": _build_nc_v4, "raw5": _build_nc_v5, "tile": _build_nc_tile}
KERNEL_VARIANT = os.environ.get("BASS_VARIANT", "raw5")


def kernel(query=None, key=None, value=None, q_param=None, _trace=False):
    from concourse.bass_utils import run_bass_kernel_spmd

    global LAST_RESULT

    value = np.ascontiguousarray(np.asarray(value, dtype=np.float32))
    assert value.shape == (B, SK, DV), value.shape

    nc = _BUILDERS[KERNEL_VARIANT]()
    shards = value.reshape(N_CORES, BPC, SK, DV)
    in_maps = [{"value": shards[i]} for i in range(N_CORES)]

    LAST_RESULT = run_bass_kernel_spmd(
        nc, in_maps, list(range(N_CORES)), trace=_trace
    )
    return np.concatenate(
        [LAST_RESULT.results[i]["out"] for i in range(N_CORES)], axis=0
    )


# revision 19
# speedup vs baseline: 1.0834x; 1.0061x over previous
"""Trainium2 Bass kernel for nn_Attention_39676907884025.

Reference semantics: q_param (a scalar) is broadcast over both query and key,
so the score matrix qk[b,q,k] = sum_d p*p is CONSTANT along the softmax axis.
Softmax of a constant row is exactly uniform (x - max(x) == 0 bit-exactly,
exp(0) == 1, sum == SK exactly, 1/SK is a power of two), so

    out[b, q, :] = (1/SK) * sum_k value[b, k, :]     for every q.

query / key / q_param never need to touch the device.

Distribution: data-parallel over batch B=16 across 8 NeuronCores (2 batches
per core). Per core the kernel is pure HBM streaming: read value (2MB),
write out (2MB); HBM-per-NC is ~358 GB/s, so the data floor is ~11.3us.

v3 (raw bacc, minimal program): three DMA queues (sync+scalar HWDGE,
gpsimd SWDGE), each warmed with a tiny first DMA (SWDGE otherwise has a
~3us cold start). Loads use the xt[p, t*128+d] = V[p*16+t, d] layout split
5/6/5 row-tiles per queue per batch. DVE tensor_reduces the sync/scalar
chunks as they land; gpsimd tensor_reduces its own chunk; two DVE adds
fold the three partials, the last one casting to bf16. One single-pass
bf16 matmul per batch against a constant 1/2048 stationary tile reduces
across partitions and broadcasts to all 128 rows; ACT copies PSUM out and
replicates to a 512-col wide tile; stores (2KB elems) go out 6/4/6 tiles
per queue, b0's stores overlapping b1's loads.

The profiler executes the NEFF twice, so every semaphore is cleared at a
provably-quiescent point at the end of each execution (each DMA queue
clears its own sems after its store-drain wait; vector clears the compute
sems after a 3-way s_done rendezvous). Without this, execution 2 starts
with stale semaphore values and races.
"""

import os
import sys

import numpy as np

if "/opt/trn_rl_repo" not in sys.path:
    sys.path.insert(0, "/opt/trn_rl_repo")

B, SQ, SK, D, DV = 16, 2048, 2048, 128, 128
N_CORES = 8
BPC = B // N_CORES  # batches per core
P = 128

LAST_RESULT = None  # BassKernelResults of the most recent run (for profiling)


def _build_nc_v3():
    import concourse.bacc as bacc
    import concourse.mybir as mybir

    WARM = os.environ.get("V3_WARM", "1") == "1"
    REDUCE = os.environ.get("V3_REDUCE", "1") == "1"
    BF16 = os.environ.get("V3_BF16", "1") == "1"
    ACT = os.environ.get("V3_ACT", "1") == "1"
    CLEAR = os.environ.get("V3_CLEAR", "1") == "1"

    f32 = mybir.dt.float32
    bf16 = mybir.dt.bfloat16 if BF16 else mybir.dt.float32
    nc = bacc.Bacc("TRN2", target_bir_lowering=False)

    val = nc.dram_tensor("value", [BPC, SK, DV], f32, kind="ExternalInput")
    out = nc.dram_tensor("out", [BPC, SQ, DV], f32, kind="ExternalOutput")

    w = nc.alloc_sbuf_tensor("w_const", [P, P], bf16)
    warm = nc.alloc_sbuf_tensor("warm", [4, DV], f32)
    xts = [nc.alloc_sbuf_tensor(f"xt{b}", [P, SK], f32) for b in range(BPC)]
    rSs = [nc.alloc_sbuf_tensor(f"rS{b}", [P, P], f32) for b in range(BPC)]
    rAs = [nc.alloc_sbuf_tensor(f"rA{b}", [P, P], f32) for b in range(BPC)]
    rGs = [nc.alloc_sbuf_tensor(f"rG{b}", [P, P], f32) for b in range(BPC)]
    sc1s = [nc.alloc_sbuf_tensor(f"sc1{b}", [P, 384], f32) for b in range(BPC)]
    cs = [nc.alloc_sbuf_tensor(f"c{b}", [P, P], f32) for b in range(BPC)]
    t4s = [nc.alloc_sbuf_tensor(f"t4_{b}", [P, P], bf16) for b in range(BPC)]
    wides = [nc.alloc_sbuf_tensor(f"wide{b}", [P, 512], f32) for b in range(BPC)]
    pss = [nc.alloc_psum_tensor(f"ps{b}", [P, P], f32) for b in range(BPC)]

    s_ls = nc.alloc_semaphore("s_ls")
    s_la = nc.alloc_semaphore("s_la")
    s_lg = nc.alloc_semaphore("s_lg")
    s_dve = nc.alloc_semaphore("s_dve")
    s_mm = nc.alloc_semaphore("s_mm")
    s_wide = nc.alloc_semaphore("s_wide")
    s_ss = nc.alloc_semaphore("s_ss")
    s_sa = nc.alloc_semaphore("s_sa")
    s_sg = nc.alloc_semaphore("s_sg")
    # cleared (gpsimd, start of every execution) in upstream-first order:
    # compute sems first so no in-flight increment can land after its clear
    _ALL_SEMS = [s_mm, s_dve, s_wide, s_ls, s_la, s_lg, s_ss, s_sa, s_sg]

    WOFF = 16 if WARM else 0  # warm DMA shifts load-sem thresholds

    # load tile split per batch: sync t[0:5), scalar t[5:11), gp t[11:16)
    # store tile split per batch: sync t[0:6), scalar t[6:10), gp t[10:16)

    def xsb(b):
        return xts[b][:].rearrange("p (t d) -> p t d", d=DV)

    def xdr(b):
        return val[b].rearrange("(p t) d -> p t d", p=P)

    def odr(b):
        return out[b].rearrange("(p t) d -> p t d", p=P)

    def wq(b):
        return wides[b][:].rearrange("p (q d) -> p q d", d=DV)

    def load(eng, b, t0, t1, sem):
        return eng.dma_start(
            xsb(b)[:, t0:t1, :], xdr(b)[:, t0:t1, :]
        ).then_inc(sem, 16)

    def chunk_view(b, t0, t1):
        return (
            xts[b][:, t0 * DV : t1 * DV]
            .rearrange("p (t d) -> p d t", d=DV)
        )

    with nc.Block() as block:

        @block.sync
        def _(sync):
            if WARM:
                sync.dma_start(warm[0:1, :], val[0, 0:1, :]).then_inc(s_ls, 16)
            load(sync, 0, 0, 5, s_ls)
            load(sync, 1, 0, 5, s_ls)
            sync.wait_ge(s_wide, 1)
            sync.dma_start(odr(0)[:, 0:4, :], wq(0)).then_inc(s_ss, 16)
            sync.dma_start(odr(0)[:, 4:6, :], wq(0)[:, 0:2, :]).then_inc(s_ss, 16)
            sync.wait_ge(s_wide, 2)
            sync.dma_start(odr(1)[:, 0:4, :], wq(1)).then_inc(s_ss, 16)
            sync.dma_start(odr(1)[:, 4:6, :], wq(1)[:, 0:2, :]).then_inc(s_ss, 16)
            sync.wait_ge(s_ss, 64)

        @block.scalar
        def _(scalar):
            if WARM:
                scalar.dma_start(warm[1:2, :], val[0, 1:2, :]).then_inc(s_la, 16)
            load(scalar, 0, 5, 11, s_la)
            load(scalar, 1, 5, 11, s_la)
            for b in range(BPC):
                if ACT:
                    scalar.wait_ge(s_mm, b + 1)
                    scalar.copy(wides[b][:, 0:P], pss[b][:])
                    scalar.copy(wides[b][:, P : 2 * P], wides[b][:, 0:P])
                    scalar.copy(
                        wides[b][:, 2 * P : 4 * P], wides[b][:, 0 : 2 * P]
                    ).then_inc(s_wide, 1)
                else:
                    scalar.wait_ge(s_wide, b + 1)
                scalar.dma_start(odr(b)[:, 6:10, :], wq(b)).then_inc(s_sa, 16)
            scalar.wait_ge(s_sa, 32)

        @block.gpsimd
        def _(gpsimd):
            if WARM:
                gpsimd.dma_start(warm[2:3, :], val[0, 2:3, :]).then_inc(s_lg, 16)
            if CLEAR:
                for s in _ALL_SEMS:
                    gpsimd.sem_clear(s)
            load(gpsimd, 0, 11, 16, s_lg)
            load(gpsimd, 1, 11, 16, s_lg)
            for b in range(BPC):
                gpsimd.wait_ge(s_wide, b + 1)
                gpsimd.dma_start(
                    odr(b)[:, 10:14, :], wq(b)
                ).then_inc(s_sg, 16)
                gpsimd.dma_start(
                    odr(b)[:, 14:16, :], wq(b)[:, 0:2, :]
                ).then_inc(s_sg, 16)
            gpsimd.wait_ge(s_sg, 64)

        @block.vector
        def _(vector):
            vector.memset(w[:], 1.0 / SK)

            def red(dst, b, t0, t1):
                if REDUCE:
                    vector.tensor_reduce(
                        dst[:],
                        chunk_view(b, t0, t1),
                        axis=mybir.AxisListType.X,
                        op=mybir.AluOpType.add,
                    )
                else:
                    x = xts[b]
                    lo, n = t0 * DV, t1 - t0
                    s1 = sc1s[b]
                    if n == 5:
                        vector.tensor_add(
                            s1[:, 0:256], x[:, lo : lo + 256],
                            x[:, lo + 256 : lo + 512],
                        )
                        vector.tensor_add(
                            dst[:], s1[:, 0:128], s1[:, 128:256]
                        )
                        vector.tensor_add(
                            dst[:], dst[:], x[:, lo + 512 : lo + 640]
                        )
                    else:  # n == 6
                        vector.tensor_add(
                            s1[:, 0:384], x[:, lo : lo + 384],
                            x[:, lo + 384 : lo + 768],
                        )
                        vector.tensor_add(
                            dst[:], s1[:, 0:128], s1[:, 128:256]
                        )
                        vector.tensor_add(
                            dst[:], dst[:], s1[:, 256:384]
                        )

            for b in range(BPC):
                th = WOFF + 16 * (b + 1)
                vector.wait_ge(s_ls, th)
                red(rSs[b], b, 0, 5)
                vector.wait_ge(s_la, th)
                red(rAs[b], b, 5, 11)
                vector.tensor_add(cs[b][:], rSs[b][:], rAs[b][:])
                vector.wait_ge(s_lg, th)
                red(rGs[b], b, 11, 16)
                vector.tensor_add(t4s[b][:], cs[b][:], rGs[b][:]).then_inc(
                    s_dve, 1
                )
                if not ACT:
                    vector.wait_ge(s_mm, b + 1)
                    vector.tensor_copy(wides[b][:, 0:P], pss[b][:])
                    vector.tensor_copy(wides[b][:, P : 2 * P], wides[b][:, 0:P])
                    vector.tensor_copy(
                        wides[b][:, 2 * P : 4 * P], wides[b][:, 0 : 2 * P]
                    ).then_inc(s_wide, 1)

        @block.tensor
        def _(tensor):
            # w readiness rides on s_dve: vector memsets w before its reduces
            for b in range(BPC):
                tensor.wait_ge(s_dve, b + 1)
                nc.tensor.matmul(
                    pss[b][:], w[:], t4s[b][:], start=True, stop=True
                ).then_inc(s_mm, 1)

    nc.compile()
    return nc


def _build_nc_v4():
    """2 HWDGE queues carry all data (SWDGE crawls at ~54-100 GB/s here);
    DVE folds all 16 row-tiles per batch (wide adds as chunks land, bf16
    cast on the last), one single-pass bf16 matmul per batch reduces
    across partitions and broadcasts; DVE replicates PSUM out to a
    512-col wide tile; each queue stores 2x4 tiles per batch.

    Semaphore discipline: a DMA's then_inc(sem, 16) arrives as +1 from each
    of the 16 SDMA engines, so with several DMAs on one semaphore a
    cumulative threshold can be met while an earlier DMA still has
    descriptors in flight on a lagging engine. Every waited-on load DMA
    therefore gets its OWN semaphore, waited at its full value (16); store
    semaphores are shared per queue because only the final total (64) is
    ever waited on. gpsimd zeroes all semaphores at the start of every
    execution (the profiler runs the NEFF twice; stale semaphores would
    race execution 2 -- and sem_inc/EventSemaphore updates from sync/scalar
    crash this runtime, so gpsimd sem_clear is the only proven mechanism).
    """
    import concourse.bacc as bacc
    import concourse.mybir as mybir

    f32 = mybir.dt.float32
    bf16 = mybir.dt.bfloat16
    nc = bacc.Bacc("TRN2", target_bir_lowering=False)

    val = nc.dram_tensor("value", [BPC, SK, DV], f32, kind="ExternalInput")
    out = nc.dram_tensor("out", [BPC, SQ, DV], f32, kind="ExternalOutput")

    w = nc.alloc_sbuf_tensor("w_const", [P, P], bf16)
    warm = nc.alloc_sbuf_tensor("warm", [4, DV], f32)
    xts = [nc.alloc_sbuf_tensor(f"xt{b}", [P, SK], f32) for b in range(BPC)]
    h1s = [nc.alloc_sbuf_tensor(f"h1_{b}", [P, 384], f32) for b in range(BPC)]
    h2s = [nc.alloc_sbuf_tensor(f"h2_{b}", [P, 384], f32) for b in range(BPC)]
    h3s = [nc.alloc_sbuf_tensor(f"h3_{b}", [P, 384], f32) for b in range(BPC)]
    h4s = [nc.alloc_sbuf_tensor(f"h4_{b}", [P, P], f32) for b in range(BPC)]
    h5s = [nc.alloc_sbuf_tensor(f"h5_{b}", [P, P], f32) for b in range(BPC)]
    pts = [nc.alloc_sbuf_tensor(f"pt{b}", [P, P], bf16) for b in range(BPC)]
    wides = [nc.alloc_sbuf_tensor(f"wide{b}", [P, 512], f32) for b in range(BPC)]
    pss = [nc.alloc_psum_tensor(f"ps{b}", [P, P], f32) for b in range(BPC)]

    s_wm = nc.alloc_semaphore("s_wm")  # warm DMAs; never waited on
    sA = [nc.alloc_semaphore(f"sA{b}") for b in range(BPC)]  # sync t[0:6)
    sB = [nc.alloc_semaphore(f"sB{b}") for b in range(BPC)]  # sync t[12:14)
    sC = [nc.alloc_semaphore(f"sC{b}") for b in range(BPC)]  # scalar t[6:12)
    sD = [nc.alloc_semaphore(f"sD{b}") for b in range(BPC)]  # scalar t[14:16)
    s_dve = nc.alloc_semaphore("s_dve")  # pt[b] ready
    s_mm = nc.alloc_semaphore("s_mm")    # ps[b] complete
    s_wide = nc.alloc_semaphore("s_wide")
    s_ss = nc.alloc_semaphore("s_ss")    # sync stores (final 64)
    s_sa = nc.alloc_semaphore("s_sa")    # scalar stores (final 64)
    _ALL_SEMS = (
        [s_mm, s_dve, s_wide] + sA + sB + sC + sD + [s_wm, s_ss, s_sa]
    )

    # stores: sync t[0:4) + t[4:8); scalar t[8:12) + t[12:16)

    def xsb(b):
        return xts[b][:].rearrange("p (t d) -> p t d", d=DV)

    def xdr(b):
        return val[b].rearrange("(p t) d -> p t d", p=P)

    def odr(b):
        return out[b].rearrange("(p t) d -> p t d", p=P)

    def wq(b):
        return wides[b][:].rearrange("p (q d) -> p q d", d=DV)

    def load(eng, b, t0, t1, sem):
        return eng.dma_start(
            xsb(b)[:, t0:t1, :], xdr(b)[:, t0:t1, :]
        ).then_inc(sem, 16)

    with nc.Block() as block:

        @block.sync
        def _(sync):
            sync.dma_start(warm[0:1, :], val[0, 0:1, :]).then_inc(s_wm, 16)
            load(sync, 0, 0, 6, sA[0])
            load(sync, 0, 12, 14, sB[0])
            load(sync, 1, 0, 6, sA[1])
            load(sync, 1, 12, 14, sB[1])
            for b in range(BPC):
                sync.wait_ge(s_wide, b + 1)
                sync.dma_start(odr(b)[:, 0:4, :], wq(b)).then_inc(s_ss, 16)
                sync.dma_start(odr(b)[:, 4:8, :], wq(b)).then_inc(s_ss, 16)
            sync.wait_ge(s_ss, 64)

        @block.scalar
        def _(scalar):
            scalar.dma_start(warm[1:2, :], val[0, 1:2, :]).then_inc(s_wm, 16)
            load(scalar, 0, 6, 12, sC[0])
            load(scalar, 0, 14, 16, sD[0])
            load(scalar, 1, 6, 12, sC[1])
            load(scalar, 1, 14, 16, sD[1])
            for b in range(BPC):
                scalar.wait_ge(s_wide, b + 1)
                scalar.dma_start(odr(b)[:, 8:12, :], wq(b)).then_inc(s_sa, 16)
                scalar.dma_start(odr(b)[:, 12:16, :], wq(b)).then_inc(s_sa, 16)
            scalar.wait_ge(s_sa, 64)

        @block.gpsimd
        def _(gpsimd):
            for s in _ALL_SEMS:
                gpsimd.sem_clear(s)

        @block.vector
        def _(vector):
            vector.memset(w[:], 1.0 / SK)
            for b in range(BPC):
                x = xts[b]
                vector.wait_ge(sA[b], 16)
                vector.tensor_add(h1s[b][:], x[:, 0:384], x[:, 384:768])
                vector.wait_ge(sC[b], 16)
                vector.tensor_add(h2s[b][:], x[:, 768:1152], x[:, 1152:1536])
                vector.tensor_add(h3s[b][:], h1s[b][:], h2s[b][:])
                vector.tensor_add(
                    h4s[b][:], h3s[b][:, 0:128], h3s[b][:, 128:256]
                )
                vector.tensor_add(h5s[b][:], h4s[b][:], h3s[b][:, 256:384])
                # tail tiles 12..15
                vector.wait_ge(sB[b], 16)
                vector.tensor_add(h4s[b][:], x[:, 1536:1664], x[:, 1664:1792])
                vector.tensor_add(h4s[b][:], h4s[b][:], h5s[b][:])
                vector.wait_ge(sD[b], 16)
                vector.tensor_add(h4s[b][:], h4s[b][:], x[:, 1792:1920])
                vector.tensor_add(
                    pts[b][:], h4s[b][:], x[:, 1920:2048]
                ).then_inc(s_dve, 1)
                vector.wait_ge(s_mm, b + 1)
                vector.tensor_copy(wides[b][:, 0:P], pss[b][:])
                vector.tensor_copy(wides[b][:, P : 2 * P], wides[b][:, 0:P])
                vector.tensor_copy(
                    wides[b][:, 2 * P : 4 * P], wides[b][:, 0 : 2 * P]
                ).then_inc(s_wide, 1)

        @block.tensor
        def _(tensor):
            # w readiness rides on s_dve: vector memsets w before its folds
            for b in range(BPC):
                tensor.wait_ge(s_dve, b + 1)
                nc.tensor.matmul(
                    pss[b][:], w[:], pts[b][:], start=True, stop=True
                ).then_inc(s_mm, 1)

    nc.compile()
    return nc


def _build_nc_v6():
    """v4 loads + merged broadcast stores (+ optional gp early slice).

    Loads (per-DMA semaphores; see v4 docstring): sync q1: b0 t[0:6),
    b0 t[12:14), b1 t[0:6), b1 t[12:14); scalar q10 (ACT ring, ~3us cold
    start): warm, b0 t[6:12), b0 t[14:16), b1 t[6:12), b1 t[14:16).
    With V6_GP=1 the gpsimd/SWDGE queue (slow but starts while q10 is
    cold) takes b0 t[12:16) instead, shrinking the HWDGE load bytes.

    DVE folds 16 tiles/batch (wide adds, bf16 cast last), one bf16 matmul
    reduces across partitions + broadcasts, 3 DVE copies replicate PSUM to
    a 512-col wide tile. Each queue stores 512KB per batch in ONE DMA:
    dst t[0:8) / t[8:16), src = wide viewed [128,4,128] broadcast to
    [128,2,4,128] (stride-0 outer; elems stay 2KB so streaming is not
    descriptor-bound, unlike a 64KB-source broadcast which generates 512B
    packets and caps near 290 GB/s). Store descriptors pre-stage while
    loads drain, making the load->store transition seamless.
    gpsimd zeroes all semaphores at execution start (profiler double-run).
    """
    import concourse.bacc as bacc
    import concourse.mybir as mybir

    GP = os.environ.get("V6_GP", "0") == "1"

    f32 = mybir.dt.float32
    bf16 = mybir.dt.bfloat16
    nc = bacc.Bacc("TRN2", target_bir_lowering=False)

    val = nc.dram_tensor("value", [BPC, SK, DV], f32, kind="ExternalInput")
    out = nc.dram_tensor("out", [BPC, SQ, DV], f32, kind="ExternalOutput")

    w = nc.alloc_sbuf_tensor("w_const", [P, P], bf16)
    warm = nc.alloc_sbuf_tensor("warm", [4, DV], f32)
    xts = [nc.alloc_sbuf_tensor(f"xt{b}", [P, SK], f32) for b in range(BPC)]
    h1s = [nc.alloc_sbuf_tensor(f"h1_{b}", [P, 384], f32) for b in range(BPC)]
    h2s = [nc.alloc_sbuf_tensor(f"h2_{b}", [P, 384], f32) for b in range(BPC)]
    h3s = [nc.alloc_sbuf_tensor(f"h3_{b}", [P, 384], f32) for b in range(BPC)]
    h4s = [nc.alloc_sbuf_tensor(f"h4_{b}", [P, P], f32) for b in range(BPC)]
    h5s = [nc.alloc_sbuf_tensor(f"h5_{b}", [P, P], f32) for b in range(BPC)]
    pts = [nc.alloc_sbuf_tensor(f"pt{b}", [P, P], bf16) for b in range(BPC)]
    wides = [nc.alloc_sbuf_tensor(f"wide{b}", [P, 512], f32) for b in range(BPC)]
    pss = [nc.alloc_psum_tensor(f"ps{b}", [P, P], f32) for b in range(BPC)]

    s_wm = nc.alloc_semaphore("s_wm")  # warm DMAs; never waited on
    sA = [nc.alloc_semaphore(f"sA{b}") for b in range(BPC)]  # sync t[0:6)
    sB = [nc.alloc_semaphore(f"sB{b}") for b in range(BPC)]  # sync t[12:14)
    sC = [nc.alloc_semaphore(f"sC{b}") for b in range(BPC)]  # scalar t[6:12)
    sD = [nc.alloc_semaphore(f"sD{b}") for b in range(BPC)]  # scalar t[14:16)
    s_dve = nc.alloc_semaphore("s_dve")
    s_mm = nc.alloc_semaphore("s_mm")
    s_wide = nc.alloc_semaphore("s_wide")
    s_ss = nc.alloc_semaphore("s_ss")    # sync stores (final 32)
    s_sa = nc.alloc_semaphore("s_sa")    # scalar stores (final 32)
    _ALL_SEMS = (
        [s_mm, s_dve, s_wide] + sA + sB + sC + sD + [s_wm, s_ss, s_sa]
    )

    def xsb(b):
        return xts[b][:].rearrange("p (t d) -> p t d", d=DV)

    def xdr(b):
        return val[b].rearrange("(p t) d -> p t d", p=P)

    def odr(b):
        return out[b].rearrange("(p t) d -> p t d", p=P)

    def load(eng, b, t0, t1, sem):
        return eng.dma_start(
            xsb(b)[:, t0:t1, :], xdr(b)[:, t0:t1, :]
        ).then_inc(sem, 16)

    def store(eng, b, t0, sem):
        src = (
            wides[b][:]
            .rearrange("p (q d) -> p q d", d=DV)
            .unsqueeze(1)
            .broadcast_to([P, 2, 4, DV])
        )
        dst = odr(b)[:, t0 : t0 + 8, :].rearrange(
            "p (a q) d -> p a q d", q=4
        )
        return eng.dma_start(dst, src).then_inc(sem, 16)

    with nc.Block() as block:

        @block.sync
        def _(sync):
            load(sync, 0, 0, 6, sA[0])
            if not GP:
                load(sync, 0, 12, 14, sB[0])
            load(sync, 1, 0, 6, sA[1])
            load(sync, 1, 12, 14, sB[1])
            for b in range(BPC):
                sync.wait_ge(s_wide, b + 1)
                store(sync, b, 0, s_ss)
            sync.wait_ge(s_ss, 32)

        @block.scalar
        def _(scalar):
            scalar.dma_start(warm[1:2, :], val[0, 1:2, :]).then_inc(s_wm, 16)
            load(scalar, 0, 6, 12, sC[0])
            if not GP:
                load(scalar, 0, 14, 16, sD[0])
            load(scalar, 1, 6, 12, sC[1])
            load(scalar, 1, 14, 16, sD[1])
            for b in range(BPC):
                scalar.wait_ge(s_wide, b + 1)
                store(scalar, b, 8, s_sa)
            scalar.wait_ge(s_sa, 32)

        @block.gpsimd
        def _(gpsimd):
            for s in _ALL_SEMS:
                gpsimd.sem_clear(s)
            if GP:
                # b0 t[12:16) on the SWDGE queue: slow, but it streams
                # while the ACT ring is still cold; incs both tail sems.
                load(gpsimd, 0, 12, 14, sB[0])
                load(gpsimd, 0, 14, 16, sD[0])

        @block.vector
        def _(vector):
            vector.memset(w[:], 1.0 / SK)
            for b in range(BPC):
                x = xts[b]
                vector.wait_ge(sA[b], 16)
                vector.tensor_add(h1s[b][:], x[:, 0:384], x[:, 384:768])
                vector.wait_ge(sC[b], 16)
                vector.tensor_add(h2s[b][:], x[:, 768:1152], x[:, 1152:1536])
                vector.tensor_add(h3s[b][:], h1s[b][:], h2s[b][:])
                vector.tensor_add(
                    h4s[b][:], h3s[b][:, 0:128], h3s[b][:, 128:256]
                )
                vector.tensor_add(h5s[b][:], h4s[b][:], h3s[b][:, 256:384])
                # tail tiles 12..15
                vector.wait_ge(sB[b], 16)
                vector.tensor_add(h4s[b][:], x[:, 1536:1664], x[:, 1664:1792])
                vector.tensor_add(h4s[b][:], h4s[b][:], h5s[b][:])
                vector.wait_ge(sD[b], 16)
                vector.tensor_add(h4s[b][:], h4s[b][:], x[:, 1792:1920])
                vector.tensor_add(
                    pts[b][:], h4s[b][:], x[:, 1920:2048]
                ).then_inc(s_dve, 1)
                vector.wait_ge(s_mm, b + 1)
                vector.tensor_copy(wides[b][:, 0:P], pss[b][:])
                vector.tensor_copy(wides[b][:, P : 2 * P], wides[b][:, 0:P])
                vector.tensor_copy(
                    wides[b][:, 2 * P : 4 * P], wides[b][:, 0 : 2 * P]
                ).then_inc(s_wide, 1)

        @block.tensor
        def _(tensor):
            # w readiness rides on s_dve: vector memsets w before its folds
            for b in range(BPC):
                tensor.wait_ge(s_dve, b + 1)
                nc.tensor.matmul(
                    pss[b][:], w[:], pts[b][:], start=True, stop=True
                ).then_inc(s_mm, 1)

    nc.compile()
    return nc


def _build_nc_tile():
    """Tile-scheduled fallback (the 28.3us baseline)."""
    import concourse.bacc as bacc
    import concourse.mybir as mybir
    from concourse.tile import TileContext

    f32 = mybir.dt.float32
    nc = bacc.Bacc("TRN2", target_bir_lowering=False)

    val = nc.dram_tensor("value", [BPC, SK, DV], f32, kind="ExternalInput")
    out = nc.dram_tensor("out", [BPC, SQ, DV], f32, kind="ExternalOutput")

    with TileContext(nc) as tc:
        with (
            tc.tile_pool(name="x", bufs=3) as xpool,
            tc.tile_pool(name="tree", bufs=3) as tpool,
            tc.tile_pool(name="const", bufs=1) as cpool,
            tc.tile_pool(name="psum", bufs=4, space="PSUM") as ppool,
        ):
            w = cpool.tile([P, P], f32)
            nc.vector.memset(w[:], 1.0 / SK)
            dma_eng = [nc.sync, nc.scalar]

            for b in range(BPC):
                xt = xpool.tile([P, SK], f32)
                xdst = xt[:].rearrange("p (t d) -> p t d", d=DV)
                xsrc = val[b].rearrange("(p t) d -> p t d", p=P)

                accs = []
                for qi in range(4):
                    t0, t1 = 4 * qi, 4 * (qi + 1)
                    dma_eng[qi % 2].dma_start(
                        xdst[:, t0:t1, :], xsrc[:, t0:t1, :]
                    )
                    lo, hi = 512 * qi, 512 * (qi + 1)
                    a = tpool.tile([P, 256], f32, tag=f"a{qi % 2}")
                    nc.vector.tensor_add(
                        a[:], xt[:, lo : lo + 256], xt[:, lo + 256 : hi]
                    )
                    acc = tpool.tile([P, P], f32, tag=f"acc{qi}")
                    nc.vector.tensor_add(acc[:], a[:, 0:128], a[:, 128:256])
                    accs.append(acc)

                s01 = tpool.tile([P, P], f32, tag="s01")
                nc.vector.tensor_add(s01[:], accs[0][:], accs[1][:])
                s23 = tpool.tile([P, P], f32, tag="s23")
                nc.vector.tensor_add(s23[:], accs[2][:], accs[3][:])
                t4 = tpool.tile([P, P], f32, tag="t4")
                nc.vector.tensor_add(t4[:], s01[:], s23[:])

                ps = ppool.tile([P, P], f32)
                nc.tensor.matmul(ps[:], w[:], t4[:], start=True, stop=True)

                wide = xpool.tile([P, 512], f32, tag="wide")
                nc.vector.tensor_copy(wide[:, 0:P], ps[:])
                nc.vector.tensor_copy(wide[:, P : 2 * P], wide[:, 0:P])
                nc.vector.tensor_copy(wide[:, 2 * P : 4 * P], wide[:, 0 : 2 * P])

                odst = out[b].rearrange("(p t) d -> p t d", p=P)
                wsrc = wide[:].rearrange("p (t d) -> p t d", d=DV)
                for qi in range(4):
                    t0, t1 = 4 * qi, 4 * (qi + 1)
                    dma_eng[qi % 2].dma_start(odst[:, t0:t1, :], wsrc)

    nc.compile()
    return nc


_BUILDERS = {"raw3": _build_nc_v3, "raw4": _build_nc_v4, "raw6": _build_nc_v6, "tile": _build_nc_tile}
KERNEL_VARIANT = os.environ.get("BASS_VARIANT", "raw6")


def kernel(query=None, key=None, value=None, q_param=None, _trace=False):
    from concourse.bass_utils import run_bass_kernel_spmd

    global LAST_RESULT

    value = np.ascontiguousarray(np.asarray(value, dtype=np.float32))
    assert value.shape == (B, SK, DV), value.shape

    nc = _BUILDERS[KERNEL_VARIANT]()
    shards = value.reshape(N_CORES, BPC, SK, DV)
    in_maps = [{"value": shards[i]} for i in range(N_CORES)]

    LAST_RESULT = run_bass_kernel_spmd(
        nc, in_maps, list(range(N_CORES)), trace=_trace
    )
    return np.concatenate(
        [LAST_RESULT.results[i]["out"] for i in range(N_CORES)], axis=0
    )
